# revision 22
# baseline (speedup 1.0000x reference)
"""AttentiveTransformer (linear -> ghost BN -> prior mask -> sparsemax) on 8 TRN2 cores.

Layout: batch rows on partitions, G=2048 on free axis. Each [128, 2048] tile is
exactly one ghost-BN chunk. Per core: batch shard of 8192 rows = 64 tiles.

Math per chunk c:
  f~  = f - colmean(f)                      (centers x since matmul is linear)
  x~  = f~ @ W.T                            (f32r matmuls, PE)
  var = sum_p(x~^2)/128                     (ones-matmul over squared output)
  a   = gamma * rsqrt(var + eps)
  z   = x~ * a * priors
  out = relu(z - tau(z)),  tau = max_r (cumsum(top16(z))_r - 1)/r
"""
import numpy as np
from contextlib import ExitStack

import concourse.bass as bass
import concourse.bacc as bacc
import concourse.tile as tile
from concourse import mybir
from concourse.bass_utils import run_bass_kernel_spmd
import concourse.bass_isa as bass_isa

F32 = mybir.dt.float32
F32R = mybir.dt.float32r
BN_EPS = 1e-5
NEG_BIG = -1.0e30

B_FULL, IN, G = 65536, 512, 2048
N_CORES = 8
P = 128
NT = G // 512          # 4 n-tiles of 512
KT = IN // 128         # 4 k-tiles of 128


def build(n_tiles, gamma_trivial, beta_zero):
    nc = bacc.Bacc()
    rows = n_tiles * P
    feat_d = nc.dram_tensor("feat", [rows, IN], F32, kind="ExternalInput")
    priors_d = nc.dram_tensor("priors", [rows, G], F32, kind="ExternalInput")
    w_d = nc.dram_tensor("w", [G, IN], F32, kind="ExternalInput")
    ident_d = nc.dram_tensor("ident", [P, P], F32, kind="ExternalInput")
    rinv_d = nc.dram_tensor("rinv", [P, 16], F32, kind="ExternalInput")
    gs_d = nc.dram_tensor("gs", [1, G], F32, kind="ExternalInput")
    ones_d = nc.dram_tensor("ones1", [P, 1], F32, kind="ExternalInput")
    onescol_d = nc.dram_tensor("onescol", [1, P], F32, kind="ExternalInput")
    negones_d = nc.dram_tensor("negones", [P, 1], F32, kind="ExternalInput")
    out_d = nc.dram_tensor("out", [rows, G], F32, kind="ExternalOutput")

    with tile.TileContext(nc) as tc, ExitStack() as ctx:
        singles = ctx.enter_context(tc.tile_pool(name="singles", bufs=1))
        fpool = ctx.enter_context(tc.tile_pool(name="fpool", bufs=3))
        ftpool = ctx.enter_context(tc.tile_pool(name="ftpool", bufs=4))
        xpool = ctx.enter_context(tc.tile_pool(name="xpool", bufs=10))
        sqpool = ctx.enter_context(tc.tile_pool(name="sqpool", bufs=6))
        ppool = ctx.enter_context(tc.tile_pool(name="ppool", bufs=3))
        zpool = ctx.enter_context(tc.tile_pool(name="zpool", bufs=4))
        zrpool = ctx.enter_context(tc.tile_pool(name="zrpool", bufs=2))
        smpool = ctx.enter_context(tc.tile_pool(name="smpool", bufs=8))
        arowpool = ctx.enter_context(tc.tile_pool(name="arowpool", bufs=6))
        areppool = ctx.enter_context(tc.tile_pool(name="areppool", bufs=8))
        wpool = ctx.enter_context(tc.tile_pool(name="wpool", bufs=2))
        ps_t = ctx.enter_context(tc.tile_pool(name="ps_t", bufs=1, space="PSUM"))
        ps_x = ctx.enter_context(tc.tile_pool(name="ps_x", bufs=3, space="PSUM"))
        ps_s = ctx.enter_context(tc.tile_pool(name="ps_s", bufs=2, space="PSUM"))
        ps_b = ctx.enter_context(tc.tile_pool(name="ps_b", bufs=2, space="PSUM"))

        # ---- constants ----
        ident = singles.tile([P, P], F32)
        nc.sync.dma_start(ident[:], ident_d[:])
        rinv = singles.tile([P, 16], F32)
        nc.sync.dma_start(rinv[:], rinv_d[:])
        gs = singles.tile([1, G], F32)
        nc.sync.dma_start(gs[:], gs_d[:])
        ones_f = singles.tile([P, 1], F32)
        nc.sync.dma_start(ones_f[:], ones_d[:])
        ones_r = singles.tile([P, 1], F32R)
        nc.scalar.copy(ones_r[:], ones_f[:])
        onescol_f = singles.tile([1, P], F32)
        nc.sync.dma_start(onescol_f[:], onescol_d[:])
        onescol_r = singles.tile([1, P], F32R)
        nc.scalar.copy(onescol_r[:], onescol_f[:])
        negones = singles.tile([P, 1], F32)
        nc.sync.dma_start(negones[:], negones_d[:])
        zeros16 = singles.tile([P, 16], F32)
        nc.vector.memset(zeros16[:], 0.0)
        eps_t = singles.tile([1, 1], F32)
        nc.vector.memset(eps_t[:], BN_EPS)

        # ---- W -> Wt (transposed, f32r) ----
        wt = singles.tile([P, KT, G], F32R)  # wt[i, k, g] = W[g, k*128+i]
        for gt in range(G // P):
            w_t = wpool.tile([P, IN], F32)
            nc.sync.dma_start(w_t[:], w_d[gt * P:(gt + 1) * P, :])
            for k in range(KT):
                pt = ps_t.tile([P, P], F32)
                nc.tensor.transpose(pt[:], w_t[:, k * P:(k + 1) * P], ident[:])
                nc.scalar.copy(wt[:, k, gt * P:(gt + 1) * P], pt[:])

        # ---- per-tile pipeline ----
        for c in range(n_tiles):
            f = fpool.tile([P, IN], F32)
            nc.sync.dma_start(f[:], feat_d[c * P:(c + 1) * P, :])
            ar = fpool.tile([P, IN], F32, tag="ar")
            nc.gpsimd.partition_all_reduce(ar[:], f[:], channels=P,
                                           reduce_op=bass_isa.ReduceOp.add)
            fc = fpool.tile([P, IN], F32, tag="fc")
            nc.vector.scalar_tensor_tensor(fc[:], in0=ar[:], scalar=-1.0 / P, in1=f[:],
                                           op0=mybir.AluOpType.mult,
                                           op1=mybir.AluOpType.add)
            fT = ftpool.tile([P, KT, P], F32R)
            for k in range(KT):
                pt = ps_t.tile([P, P], F32)
                nc.tensor.transpose(pt[:], fc[:, k * P:(k + 1) * P], ident[:])
                nc.scalar.copy(fT[:, k, :], pt[:])

            p_t = ppool.tile([P, G], F32)
            nc.sync.dma_start(p_t[:], priors_d[c * P:(c + 1) * P, :])

            z = zpool.tile([P, G], F32)
            for n in range(NT):
                px = ps_x.tile([P, 512], F32, tag="px")
                for k in range(KT):
                    nc.tensor.matmul(px[:], fT[:, k, :],
                                     wt[:, k, n * 512:(n + 1) * 512],
                                     start=(k == 0), stop=(k == KT - 1))
                xsq = sqpool.tile([P, 512], F32R)
                nc.scalar.activation(xsq[:], px[:], mybir.ActivationFunctionType.Square)
                # u = x~ * priors straight from PSUM (no x_sb copy)
                u = xpool.tile([P, 512], F32)
                nc.vector.tensor_tensor(u[:], px[:], p_t[:, n * 512:(n + 1) * 512],
                                        op=mybir.AluOpType.mult)
                vps = ps_s.tile([1, 512], F32)
                nc.tensor.matmul(vps[:], ones_r[:], xsq[:], start=True, stop=True)
                # srow = sqrt(var + eps) as f32r row; rank-1 PE broadcast; 1/x on DVE
                srow = arowpool.tile([1, 512], F32R, tag="arow")
                nc.scalar.activation(srow[:], vps[:], mybir.ActivationFunctionType.Sqrt,
                                     bias=eps_t[:])
                sps = ps_b.tile([P, 512], F32)
                nc.tensor.matmul(sps[:], onescol_r[:], srow[:], start=True, stop=True)
                arep = areppool.tile([P, 512], F32, tag="arep")
                nc.vector.reciprocal_approx_fast(arep[:], sps[:])
                if not gamma_trivial:
                    grow = arowpool.tile([1, 512], F32R, tag="grow")
                    nc.scalar.activation(grow[:], gs[:, n * 512:(n + 1) * 512],
                                         mybir.ActivationFunctionType.Copy)
                    gps = ps_b.tile([P, 512], F32)
                    nc.tensor.matmul(gps[:], onescol_r[:], grow[:], start=True, stop=True)
                    nc.vector.tensor_tensor(arep[:], arep[:], gps[:],
                                            op=mybir.AluOpType.mult)
                zn = z[:, n * 512:(n + 1) * 512]
                nc.gpsimd.tensor_tensor(zn, u[:], arep[:], op=mybir.AluOpType.mult)

            # top-16 + tau
            m16 = smpool.tile([P, 16], F32)
            zr = zrpool.tile([P, G], F32)
            nc.vector.max(m16[:, 0:8], z[:])
            nc.vector.match_replace(zr[:], in_to_replace=m16[:, 0:8], in_values=z[:],
                                    imm_value=NEG_BIG)
            nc.vector.max(m16[:, 8:16], zr[:])
            cs = smpool.tile([P, 16], F32)
            nc.vector.tensor_tensor_scan(cs[:], m16[:], zeros16[:], 0.0,
                                         op0=mybir.AluOpType.add,
                                         op1=mybir.AluOpType.bypass)
            taur = smpool.tile([P, 16], F32)
            nc.vector.scalar_tensor_tensor(taur[:], in0=cs[:], scalar=-1.0, in1=rinv[:],
                                           op0=mybir.AluOpType.add,
                                           op1=mybir.AluOpType.mult)
            ntau = smpool.tile([P, 1], F32)
            nc.vector.tensor_reduce(ntau[:], taur[:], axis=mybir.AxisListType.X,
                                    op=mybir.AluOpType.max, negate=True)
            for n in range(NT):
                zn = z[:, n * 512:(n + 1) * 512]
                nc.scalar.activation(zn, zn, mybir.ActivationFunctionType.Relu,
                                     bias=ntau[:])
            nc.sync.dma_start(out_d[c * P:(c + 1) * P, :], z[:])

    nc.finalize()
    return nc


def _consts():
    ident = np.eye(P, dtype=np.float32)
    rinv = np.broadcast_to(1.0 / np.arange(1, 17, dtype=np.float32), (P, 16)).copy()
    ones1 = np.full((P, 1), 1.0 / P, dtype=np.float32)
    return ident, rinv, ones1


_CACHE = {}


def kernel(priors, processed_feat, W, gamma, beta):
    priors = np.ascontiguousarray(priors, dtype=np.float32)
    feat = np.ascontiguousarray(processed_feat, dtype=np.float32)
    W = np.ascontiguousarray(W, dtype=np.float32)
    gamma = np.asarray(gamma, dtype=np.float32)
    beta = np.asarray(beta, dtype=np.float32)

    B = feat.shape[0]
    n_cores = N_CORES
    shard = B // n_cores
    n_tiles = shard // P
    gamma_trivial = bool(np.all(gamma == 1.0))
    beta_zero = bool(np.all(beta == 0.0))
    assert beta_zero, "beta != 0 path not implemented"

    key = (n_tiles, gamma_trivial, beta_zero)
    if key not in _CACHE:
        _CACHE[key] = build(*key)
    nc = _CACHE[key]

    ident, rinv, ones1 = _consts()
    onescol = np.ones((1, P), dtype=np.float32)
    negones = np.full((P, 1), -1.0 / P, dtype=np.float32)
    gs = gamma.reshape(1, G)
    in_maps = []
    for i in range(n_cores):
        in_maps.append({
            "feat": feat[i * shard:(i + 1) * shard],
            "priors": priors[i * shard:(i + 1) * shard],
            "w": W,
            "ident": ident,
            "rinv": rinv,
            "gs": gs,
            "ones1": ones1,
            "onescol": onescol,
            "negones": negones,
        })
    res = run_bass_kernel_spmd(nc, in_maps, core_ids=list(range(n_cores)))
    return np.concatenate([r["out"] for r in res.results], axis=0)
